# revision 6
# baseline (speedup 1.0000x reference)
"""Trainium2 Bass kernel for nn_DDSPGenerator: data-parallel over batch on 8 cores.

Self-contained: hardcodes all shapes. kernel(**inputs) takes the full unsharded
inputs (as in reference.setup_inputs()) and returns the full [16, 1, 16384] output.

Per-core plan (local batch = 2):
 - conv trunks as fp16 matmuls; frequency path in hi/lo-split fp16 (3 products,
   ~fp32 precision) because phase integration amplifies relative f error.
 - linear 256x upsample of f and loudness^2 via broadcast-AP lerp on DVE.
 - oscillator bank: fp32 cumsum (tensor_tensor_scan) of f/SR in cycles, phase
   carry-wrapped every 512 samples, per-element round-subtract wrap, ACT
   Sin(scale=2pi), multiply by amp, PE ones-reduction over the 128 oscillators.
 - noise bank: rfft/irfft as tiny DFT matmuls; 50%-overlap-add folded into the
   irfft matmul accumulation (shifted lhsT frame window + column-sliced rhs).
"""
import sys
for p in ('/opt/trn_rl_repo', '/root/.axon_site', '/root/.axon_site/_ro/trn_rl_repo',
          '/root/.axon_site/_ro/pypackages'):
    if p not in sys.path:
        sys.path.append(p)

import numpy as np
import concourse.bacc as bacc
import concourse.tile as tile
from concourse import mybir
from concourse.bass_utils import run_bass_kernel_spmd

F32 = mybir.dt.float32
F16 = mybir.dt.float16
AF = mybir.ActivationFunctionType
OP = mybir.AluOpType

SR = 22050.0
MAGIC = 12582912.0          # 1.5 * 2^23: add+sub rounds fp32 to nearest int
TWO_PI = float(2.0 * np.pi)
INV_SR = float(np.float32(1.0 / SR))
N_CORES = 8
BL = 2                      # batch per core
UPF = 256
OUT_T = 16384
NOSC = 128
NCF = 17
NFRM = 1024
CH = 1024                   # osc pipeline chunk (free dim)
SCH = 512                   # scan sub-chunk (phase carry-wrap granularity)
TW = 132                    # trunk tile width: b0 data 1..64, b1 data 67..130

NBW = [134, 262, 518, 1030]   # noise path input per-batch widths (T + 6)
NT = [128, 256, 512, 1024]    # noise path output T per layer
NT_F = [12, 48, 48, 48, 12]   # trunk layer weight tile counts
NT_N = [28, 112, 112, 112]    # noise layer weight tile counts

_CACHE = {}
SIM_SAFE = False   # replace Prelu (unimplemented in CoreSim) with Identity+max


def _idx(co, ci, k, nci, K):
    return ((co * nci) + ci) * K + k


def _build():
    nc = bacc.Bacc("TRN2", target_bir_lowering=False, debug=False)

    def din(name, shape, dt):
        return nc.dram_tensor(name, shape, dt, kind="ExternalInput").ap()

    xh = din("xh", [128, TW], F16)
    xl = din("xl", [128, TW], F16)
    # f-path weights: hi tiles then lo tiles in one array per layer
    wf = [din(f"wf{l}", [128, 2 * NT_F[l] * 128], F16) for l in range(5)]
    wg = [din(f"wg{l}", [128, NT_F[l] * 128], F16) for l in range(5)]
    wn = [din(f"wn{l}", [128, NT_N[l] * 128], F16) for l in range(4)]
    wnl = din("wnl", [128, 12 * NCF], F16)
    bf = din("bf", [128, 17], F32)
    bg = din("bg", [128, 17], F32)
    bn = din("bn", [128, 16], F32)
    bnl = din("bnl", [NCF, 1], F32)
    fr = din("fr", [32, 2 * NFRM], F16)
    dftC = din("dftC", [32, NCF], F16)
    dftS = din("dftS", [32, NCF], F16)
    dftCi = din("dftCi", [NCF, 32], F16)
    dftSi = din("dftSi", [NCF, 32], F16)
    wup = din("wup", [128, UPF], F32)
    ones16 = din("ones16", [128, 1], F16)
    kst = din("kst", [128, 1], F32)
    kdf = din("kdf", [128, 1], F32)

    OUT = nc.dram_tensor("out", [BL, OUT_T], F32, kind="ExternalOutput").ap()

    with tile.TileContext(nc) as tc:
        with tc.tile_pool(name="const", bufs=1) as pc, \
             tc.tile_pool(name="wpool", bufs=2) as pw, \
             tc.tile_pool(name="acts", bufs=1) as pa, \
             tc.tile_pool(name="tmp", bufs=2) as pt, \
             tc.tile_pool(name="osc", bufs=2) as po, \
             tc.tile_pool(name="cpsp", bufs=3, space="PSUM") as pps, \
             tc.tile_pool(name="hpsp", bufs=2, space="PSUM") as pph, \
             tc.tile_pool(name="opsp", bufs=2, space="PSUM") as ppo:

            def load(ap_in, shape, dt, pool=pc, tag=None):
                t = pool.tile(shape, dt, tag=tag or ("ld_" + ap_in.tensor.name))
                nc.sync.dma_start(out=t[:], in_=ap_in)
                return t

            xh_t = load(xh, [128, TW], F16)
            xl_t = load(xl, [128, TW], F16)
            bf_t = load(bf, [128, 17], F32)
            bg_t = load(bg, [128, 17], F32)
            bn_t = load(bn, [128, 16], F32)
            bnl_t = load(bnl, [NCF, 1], F32)
            fr_t = load(fr, [32, 2 * NFRM], F16)
            dC_t = load(dftC, [32, NCF], F16)
            dS_t = load(dftS, [32, NCF], F16)
            dCi_t = load(dftCi, [NCF, 32], F16)
            dSi_t = load(dftSi, [NCF, 32], F16)
            wup_t = load(wup, [128, UPF], F32)
            on_t = load(ones16, [128, 1], F16)
            kst_t = load(kst, [128, 1], F32)
            kdf_t = load(kdf, [128, 1], F32)

            # [128, 2, 64] data-col views (trunk layout)
            def dv(t):
                return t[:, 0:132].rearrange("p (b c) -> p b c", b=2)[:, :, 1:65]

            def pv(ps):
                return ps[:, 0:132].rearrange("p (b c) -> p b c", b=2)[:, :, 0:64]

            # ============ trunk: f path (hi/lo) + amp path ============
            fh_prev, fl_prev = [xh_t], [xl_t]
            ga_prev = [xh_t]
            for l in range(5):
                ntl = NT_F[l]
                nci = len(fh_prev)
                nco = 4 if l < 4 else 1
                wf_t = load(wf[l], [128, 2 * ntl * 128], F16, pw, tag="w")
                wg_t = load(wg[l], [128, ntl * 128], F16, pw, tag="w")
                fh_cur, fl_cur, ga_cur = [], [], []
                for co in range(nco):
                    # f path: 3-product hi/lo conv
                    ps = pps.tile([128, 512], F32, tag="cps")
                    n_mm = nci * 3 * 3
                    i_mm = 0
                    for ci in range(nci):
                        for k in range(3):
                            t_i = _idx(co, ci, k, nci, 3)
                            hsl = slice(t_i * 128, (t_i + 1) * 128)
                            lsl = slice((ntl + t_i) * 128, (ntl + t_i + 1) * 128)
                            for (wsl, rt) in ((hsl, fh_prev[ci]), (hsl, fl_prev[ci]),
                                              (lsl, fh_prev[ci])):
                                nc.tensor.matmul(ps[:, 0:130], lhsT=wf_t[:, wsl],
                                                 rhs=rt[:, k:k + 130],
                                                 start=(i_mm == 0),
                                                 stop=(i_mm == n_mm - 1))
                                i_mm += 1
                    if l < 4:
                        a32 = pt.tile([128, TW], F32, tag="t32")
                        if SIM_SAFE:
                            nc.scalar.activation(dv(a32)[:, :, :], pv(ps)[:, :, :],
                                                 AF.Identity,
                                                 bias=bf_t[:, 4 * l + co:4 * l + co + 1])
                            nc.vector.scalar_tensor_tensor(
                                out=dv(a32)[:, :, :], in0=dv(a32)[:, :, :],
                                scalar=0.2, op0=OP.mult,
                                in1=dv(a32)[:, :, :], op1=OP.max)
                        else:
                            nc.scalar.activation(dv(a32)[:, :, :], pv(ps)[:, :, :],
                                                 AF.Prelu, alpha=0.2,
                                                 bias=bf_t[:, 4 * l + co:4 * l + co + 1])
                        hi = pa.tile([128, TW], F16, tag=f"fh{l}{co}")
                        lo = pa.tile([128, TW], F16, tag=f"fl{l}{co}")
                        nc.vector.memset(hi[:], 0.0)
                        nc.vector.memset(lo[:], 0.0)
                        nc.scalar.activation(dv(hi)[:, :, :], dv(a32)[:, :, :], AF.Copy)
                        nc.vector.tensor_tensor(out=dv(lo)[:, :, :],
                                                in0=dv(a32)[:, :, :],
                                                in1=dv(hi)[:, :, :], op=OP.subtract)
                        fh_cur.append(hi)
                        fl_cur.append(lo)
                    else:
                        fsig = pa.tile([128, TW], F32, tag="fsig")
                        nc.scalar.activation(dv(fsig)[:, :, :], pv(ps)[:, :, :],
                                             AF.Sigmoid, bias=bf_t[:, 16:17])
                    # amp path: plain fp16 conv
                    ps2 = pps.tile([128, 512], F32, tag="cps")
                    i_mm = 0
                    for ci in range(nci):
                        for k in range(3):
                            t_i = _idx(co, ci, k, nci, 3)
                            nc.tensor.matmul(ps2[:, 0:130],
                                             lhsT=wg_t[:, t_i * 128:(t_i + 1) * 128],
                                             rhs=ga_prev[ci][:, k:k + 130],
                                             start=(i_mm == 0),
                                             stop=(i_mm == nci * 3 - 1))
                            i_mm += 1
                    if l < 4:
                        ah = pa.tile([128, TW], F16, tag=f"ga{l}{co}")
                        nc.vector.memset(ah[:], 0.0)
                        if SIM_SAFE:
                            tb = pt.tile([128, TW], F32, tag="t32b")
                            nc.scalar.activation(dv(tb)[:, :, :], pv(ps2)[:, :, :],
                                                 AF.Identity,
                                                 bias=bg_t[:, 4 * l + co:4 * l + co + 1])
                            nc.vector.scalar_tensor_tensor(
                                out=dv(ah)[:, :, :], in0=dv(tb)[:, :, :],
                                scalar=0.2, op0=OP.mult,
                                in1=dv(tb)[:, :, :], op1=OP.max)
                        else:
                            nc.scalar.activation(dv(ah)[:, :, :], pv(ps2)[:, :, :],
                                                 AF.Prelu, alpha=0.2,
                                                 bias=bg_t[:, 4 * l + co:4 * l + co + 1])
                        ga_cur.append(ah)
                    else:
                        amp32 = pa.tile([128, TW], F32, tag="amp32")
                        nc.scalar.activation(dv(amp32)[:, :, :], pv(ps2)[:, :, :],
                                             AF.Square, bias=bg_t[:, 16:17])
                if l < 4:
                    fh_prev, fl_prev, ga_prev = fh_cur, fl_cur, ga_cur

            # f_hz = clip(fsig*kdf + kst, 20, SR/2)
            fhz = pa.tile([128, TW], F32, tag="fhz")
            nc.vector.tensor_scalar(out=dv(fhz)[:, :, :], in0=dv(fsig)[:, :, :],
                                    scalar1=kdf_t[:], op0=OP.mult,
                                    scalar2=kst_t[:], op1=OP.add)
            nc.vector.tensor_scalar(out=dv(fhz)[:, :, :], in0=dv(fhz)[:, :, :],
                                    scalar1=20.0, op0=OP.max,
                                    scalar2=SR / 2.0, op1=OP.min)

            # ============ noise path ============
            R1 = pa.tile([128, 2 * NBW[0]], F16, tag="R1")
            nc.gpsimd.memset(R1[:], 0.0)
            src = xh_t[:, 0:132].rearrange("p (b c) -> p b c", b=2)[:, :, 1:65]
            src = src.unsqueeze(3).broadcast_to([128, 2, 64, 2])
            dst = R1[:, 0:2 * NBW[0]].rearrange("p (b c) -> p b c", b=2)[:, :, 3:131]
            dst = dst.rearrange("p b (i r) -> p b i r", r=2)
            nc.gpsimd.tensor_copy(dst, src)
            R = [[R1]]

            ncis = [1, 4, 4, 4]
            for l in range(4):
                nci = ncis[l]
                wn_t = load(wn[l], [128, NT_N[l] * 128], F16, pw, tag="w")
                Tl = NT[l]
                bw = NBW[l]
                last = (l == 3)
                obw = 1026 if last else NBW[l + 1]
                opad = 1 if last else 3
                cur = []
                for co in range(4):
                    dst_t = pa.tile([128, 2 * obw], F16, tag=f"N{l}{co}")
                    nc.gpsimd.memset(dst_t[:], 0.0)
                    for b in range(BL):
                        for h in range((Tl + 511) // 512):
                            n_out = min(512, Tl - 512 * h)
                            ps = pps.tile([128, 512], F32, tag="cps")
                            i_mm = 0
                            for ci in range(nci):
                                for k in range(7):
                                    t_i = _idx(co, ci, k, nci, 7)
                                    nc.tensor.matmul(
                                        ps[:, 0:n_out],
                                        lhsT=wn_t[:, t_i * 128:(t_i + 1) * 128],
                                        rhs=R[l][ci][:, b * bw + 512 * h + k:
                                                     b * bw + 512 * h + k + n_out],
                                        start=(i_mm == 0), stop=(i_mm == nci * 7 - 1))
                                    i_mm += 1
                            cmp16 = pt.tile([128, 512], F16, tag="ncmp")
                            if SIM_SAFE:
                                tc32 = pt.tile([128, 512], F32, tag="t32c")
                                nc.scalar.activation(tc32[:, 0:n_out], ps[:, 0:n_out],
                                                     AF.Identity,
                                                     bias=bn_t[:, 4 * l + co:
                                                               4 * l + co + 1])
                                nc.vector.scalar_tensor_tensor(
                                    out=cmp16[:, 0:n_out], in0=tc32[:, 0:n_out],
                                    scalar=0.2, op0=OP.mult,
                                    in1=tc32[:, 0:n_out], op1=OP.max)
                            else:
                                nc.scalar.activation(cmp16[:, 0:n_out], ps[:, 0:n_out],
                                                     AF.Prelu, alpha=0.2,
                                                     bias=bn_t[:, 4 * l + co:
                                                               4 * l + co + 1])
                            if last:
                                nc.gpsimd.tensor_copy(
                                    dst_t[:, b * obw + opad + 512 * h:
                                          b * obw + opad + 512 * h + n_out],
                                    cmp16[:, 0:n_out])
                            else:
                                do = dst_t[:, b * obw + opad + 1024 * h:
                                           b * obw + opad + 1024 * h + 2 * n_out]
                                do = do.rearrange("p (i r) -> p i r", r=2)
                                si = cmp16[:, 0:n_out].unsqueeze(2).broadcast_to(
                                    [128, n_out, 2])
                                nc.gpsimd.tensor_copy(do, si)
                    cur.append(dst_t)
                R.append(cur)

            # nlw -> mags16 [17, 2048]
            wnl_t = load(wnl, [128, 12 * NCF], F16, pw, tag="w")
            mags = pa.tile([NCF, 2 * NFRM], F16, tag="mags")
            H4 = R[4]
            for b in range(BL):
                for h in range(2):
                    ps = pps.tile([NCF, 512], F32, tag="cps")
                    i_mm = 0
                    for ci in range(4):
                        for k in range(3):
                            t_i = _idx(0, ci, k, 4, 3)
                            nc.tensor.matmul(
                                ps[:, 0:512],
                                lhsT=wnl_t[:, t_i * NCF:(t_i + 1) * NCF],
                                rhs=H4[ci][:, b * 1026 + 512 * h + k:
                                           b * 1026 + 512 * h + k + 512],
                                start=(i_mm == 0), stop=(i_mm == 11))
                            i_mm += 1
                    nc.scalar.activation(mags[:, b * NFRM + 512 * h:
                                              b * NFRM + 512 * h + 512],
                                         ps[:, 0:512], AF.Square, bias=bnl_t[:])

            # DFT: ReF/ImF [17, 1025] per batch (col0 zero)
            ReF = [pa.tile([NCF, NFRM + 1], F16, tag=f"ReF{b}", name=f"ReF{b}") for b in range(BL)]
            ImF = [pa.tile([NCF, NFRM + 1], F16, tag=f"ImF{b}", name=f"ImF{b}") for b in range(BL)]
            for b in range(BL):
                nc.vector.memset(ReF[b][:, 0:1], 0.0)
                nc.vector.memset(ImF[b][:, 0:1], 0.0)
                for h in range(2):
                    fsl = fr_t[:, b * NFRM + 512 * h: b * NFRM + 512 * h + 512]
                    msl = mags[:, b * NFRM + 512 * h: b * NFRM + 512 * h + 512]
                    pr = pps.tile([NCF, 512], F32, tag="cps")
                    nc.tensor.matmul(pr[:, :], lhsT=dC_t[:], rhs=fsl,
                                     start=True, stop=True)
                    nc.vector.tensor_tensor(
                        out=ReF[b][:, 1 + 512 * h: 513 + 512 * h],
                        in0=pr[:, :], in1=msl, op=OP.mult)
                    pi = pps.tile([NCF, 512], F32, tag="cps")
                    nc.tensor.matmul(pi[:, :], lhsT=dS_t[:], rhs=fsl,
                                     start=True, stop=True)
                    nc.vector.tensor_tensor(
                        out=ImF[b][:, 1 + 512 * h: 513 + 512 * h],
                        in0=pi[:, :], in1=msl, op=OP.mult)

            # noise audio, frame-major, with OLA folded into PSUM accumulation
            nv = []
            for b in range(BL):
                nvt = pa.tile([128, 128], F32, tag=f"nv{b}")
                nv.append(nvt)
                for g in range(8):
                    ps = ppo.tile([128, 16], F32, tag="ops")
                    c0 = 1 + 128 * g
                    nc.tensor.matmul(ps[:, :], lhsT=ReF[b][:, c0:c0 + 128],
                                     rhs=dCi_t[:, 0:16], start=True, stop=False)
                    nc.tensor.matmul(ps[:, :], lhsT=ImF[b][:, c0:c0 + 128],
                                     rhs=dSi_t[:, 0:16], start=False, stop=False)
                    nc.tensor.matmul(ps[:, :], lhsT=ReF[b][:, c0 - 1:c0 + 127],
                                     rhs=dCi_t[:, 16:32], start=False, stop=False)
                    nc.tensor.matmul(ps[:, :], lhsT=ImF[b][:, c0 - 1:c0 + 127],
                                     rhs=dSi_t[:, 16:32], start=False, stop=True)
                    ot = pt.tile([128, 16], F32, tag="ot")
                    nc.scalar.activation(ot[:], ps[:], AF.Copy)
                    # s = 2048g + 16f + j -> nv row s//128, col s%128
                    out_ap = nvt[16 * g:16 * (g + 1), :].rearrange(
                        "p (f2 j) -> p f2 j", j=16)
                    nc.sync.dma_start(out=out_ap, in_=ot[:])

            # ============ oscillator stage ============
            fe, dfe, ae, dae = [], [], [], []
            for b in range(BL):
                f_e = pa.tile([128, 66], F32, tag=f"fe{b}")
                nc.vector.tensor_scalar(out=f_e[:, 1:65],
                                        in0=fhz[:, 66 * b + 1:66 * b + 65],
                                        scalar1=INV_SR, op0=OP.mult,
                                        scalar2=0.0, op1=OP.add)
                nc.vector.tensor_copy(f_e[:, 0:1], f_e[:, 1:2])
                nc.vector.tensor_copy(f_e[:, 65:66], f_e[:, 64:65])
                d_f = pa.tile([128, 65], F32, tag=f"dfe{b}")
                nc.vector.tensor_tensor(out=d_f[:], in0=f_e[:, 1:66],
                                        in1=f_e[:, 0:65], op=OP.subtract)
                a_e = pa.tile([128, 66], F32, tag=f"ae{b}")
                nc.gpsimd.tensor_copy(a_e[:, 1:65], amp32[:, 66 * b + 1:66 * b + 65])
                nc.gpsimd.tensor_copy(a_e[:, 0:1], a_e[:, 1:2])
                nc.gpsimd.tensor_copy(a_e[:, 65:66], a_e[:, 64:65])
                d_a = pa.tile([128, 65], F32, tag=f"dae{b}")
                nc.gpsimd.tensor_tensor(out=d_a[:], in0=a_e[:, 1:66],
                                        in1=a_e[:, 0:65], op=OP.subtract)
                fe.append(f_e)
                dfe.append(d_f)
                ae.append(a_e)
                dae.append(d_a)

            carry, hv = [], []
            for b in range(BL):
                ct = pa.tile([128, 1], F32, tag=f"carry{b}")
                nc.vector.memset(ct[:], 0.0)
                carry.append(ct)
                hvt = pa.tile([128, 128], F32, tag=f"hv{b}")
                hv.append(hvt)

            for c in range(OUT_T // CH):
                for b in range(BL):
                    fU = po.tile([128, CH], F32, tag="fU")
                    aU = po.tile([128, CH], F32, tag="aU")
                    i0 = 4 * c
                    for (dst2, a_e, d_e) in ((fU, fe[b], dfe[b]), (aU, ae[b], dae[b])):
                        nc.vector.tensor_tensor(
                            out=dst2[:, 0:128],
                            in0=d_e[:, i0:i0 + 1].broadcast_to([128, 128]),
                            in1=wup_t[:, 128:256], op=OP.mult)
                        nc.vector.tensor_tensor(
                            out=dst2[:, 0:128], in0=dst2[:, 0:128],
                            in1=a_e[:, i0:i0 + 1].broadcast_to([128, 128]), op=OP.add)
                        d3 = d_e[:, i0 + 1:i0 + 4].unsqueeze(2).broadcast_to(
                            [128, 3, 256])
                        a3 = a_e[:, i0 + 1:i0 + 4].unsqueeze(2).broadcast_to(
                            [128, 3, 256])
                        w3 = wup_t[:].unsqueeze(1).broadcast_to([128, 3, 256])
                        m3 = dst2[:, 128:896].rearrange("p (i j) -> p i j", j=256)
                        nc.vector.tensor_tensor(out=m3, in0=d3, in1=w3, op=OP.mult)
                        nc.vector.tensor_tensor(out=m3, in0=m3, in1=a3, op=OP.add)
                        nc.vector.tensor_tensor(
                            out=dst2[:, 896:1024],
                            in0=d_e[:, i0 + 4:i0 + 5].broadcast_to([128, 128]),
                            in1=wup_t[:, 0:128], op=OP.mult)
                        nc.vector.tensor_tensor(
                            out=dst2[:, 896:1024], in0=dst2[:, 896:1024],
                            in1=a_e[:, i0 + 4:i0 + 5].broadcast_to([128, 128]),
                            op=OP.add)

                    ph = po.tile([128, CH], F32, tag="ph")
                    t1 = po.tile([128, CH], F32, tag="t1")
                    t2 = po.tile([128, CH], F32, tag="t2")
                    for q in range(CH // SCH):
                        s = slice(q * SCH, (q + 1) * SCH)
                        e = slice((q + 1) * SCH - 1, (q + 1) * SCH)
                        nc.vector.tensor_tensor_scan(
                            ph[:, s], data0=fU[:, s], data1=fU[:, s],
                            initial=carry[b][:], op0=OP.add, op1=OP.bypass)
                        nc.vector.tensor_scalar(
                            out=t1[:, 0:1], in0=ph[:, e], scalar1=-1.0, op0=OP.mult,
                            scalar2=MAGIC, op1=OP.add)
                        nc.vector.tensor_scalar_sub(out=t2[:, 0:1], in0=t1[:, 0:1],
                                                    scalar1=MAGIC)
                        nc.vector.scalar_tensor_tensor(
                            out=carry[b][:], in0=t2[:, 0:1], scalar=1.0, op0=OP.mult,
                            in1=ph[:, e], op1=OP.add)
                    nc.gpsimd.tensor_scalar(out=t1[:], in0=ph[:], scalar1=-1.0,
                                            op0=OP.mult, scalar2=MAGIC, op1=OP.add)
                    nc.gpsimd.tensor_scalar(out=t2[:], in0=t1[:], scalar1=MAGIC,
                                            op0=OP.subtract, scalar2=0.0, op1=OP.add)
                    nc.vector.scalar_tensor_tensor(out=t1[:], in0=t2[:], scalar=1.0,
                                                   op0=OP.mult, in1=ph[:], op1=OP.add)
                    nc.scalar.activation(t2[:], t1[:], AF.Sin, scale=TWO_PI)
                    p16 = po.tile([128, CH], F16, tag="p16")
                    nc.vector.tensor_tensor(out=p16[:], in0=t2[:], in1=aU[:],
                                            op=OP.mult)
                    for q in range(CH // 512):
                        hp = pph.tile([1, 512], F32, tag="hps")
                        nc.tensor.matmul(hp[:, :], lhsT=on_t[:],
                                         rhs=p16[:, q * 512:(q + 1) * 512],
                                         start=True, stop=True)
                        hs = pt.tile([1, 512], F32, tag="hs")
                        nc.scalar.activation(hs[:], hp[:], AF.Copy)
                        # s = 1024c + 512q + j -> hv rows 8c+4q .. +4
                        r0 = 8 * c + 4 * q
                        nc.sync.dma_start(out=hv[b][r0:r0 + 4, :], in_=hs[:])

            # ============ final combine ============
            for b in range(BL):
                ov = pt.tile([128, 128], F32, tag="ov")
                nc.vector.tensor_tensor(out=ov[:], in0=hv[b][:], in1=nv[b][:],
                                        op=OP.add)
                nc.sync.dma_start(out=OUT[b].rearrange("(p c) -> p c", c=128),
                                  in_=ov[:])

    nc.compile()
    return nc


# ================= host side =================

def _pack_w(w, mdim=128):
    """w [Cout, Cin, K] -> [128, ntile*mdim] partition-major lhsT pack.
    tile order ((co*nci)+ci)*K + k; lhsT[p, m] = w[co*mdim+m, ci*128+p, k]."""
    w = np.asarray(w, np.float32)
    Cout, Cin, K = w.shape
    nco, nci = Cout // mdim, Cin // 128
    ws = w.reshape(nco, mdim, nci, 128, K)
    tiles = ws.transpose(0, 2, 4, 3, 1)            # [co, ci, k, p, m]
    return np.ascontiguousarray(
        tiles.reshape(nco * nci * K, 128, mdim).transpose(1, 0, 2).reshape(128, -1))


def _split16(a):
    hi = a.astype(np.float16)
    lo = (a.astype(np.float32) - hi.astype(np.float32)).astype(np.float16)
    return hi, lo


def _prep_shared(inputs):
    g = lambda n: np.asarray(inputs[n], np.float32)
    m = {}
    bf = np.zeros((128, 17), np.float32)
    bg = np.zeros((128, 17), np.float32)
    for l, (wname, bname) in enumerate([('mw0', 'mb0'), ('mw1', 'mb1'),
                                        ('mw2', 'mb2'), ('mw3', 'mb3'),
                                        ('fw', 'fb')]):
        p = _pack_w(g(wname))
        hi, lo = _split16(p)
        m[f'wf{l}'] = np.concatenate([hi, lo], axis=1)
        b = g(bname)
        if l < 4:
            bf[:, 4 * l:4 * l + 4] = b.reshape(4, 128).T
        else:
            bf[:, 16] = b
    for l, (wname, bname) in enumerate([('gw0', 'gb0'), ('gw1', 'gb1'),
                                        ('gw2', 'gb2'), ('gw3', 'gb3'),
                                        ('lw', 'lb')]):
        m[f'wg{l}'] = _pack_w(g(wname)).astype(np.float16)
        b = g(bname)
        if l < 4:
            bg[:, 4 * l:4 * l + 4] = b.reshape(4, 128).T
        else:
            bg[:, 16] = b
    bn_arr = np.zeros((128, 16), np.float32)
    for l, (wname, bname) in enumerate([('nw0', 'nb0'), ('nw1', 'nb1'),
                                        ('nw2', 'nb2'), ('nw3', 'nb3')]):
        m[f'wn{l}'] = _pack_w(g(wname)).astype(np.float16)
        bn_arr[:, 4 * l:4 * l + 4] = g(bname).reshape(4, 128).T
    m['wnl'] = _pack_w(g('nlw'), mdim=NCF).astype(np.float16)
    m['bf'] = bf
    m['bg'] = bg
    m['bn'] = bn_arr
    m['bnl'] = np.ascontiguousarray(g('nlb').reshape(NCF, 1))

    n = np.arange(32)
    cc = np.arange(NCF)
    ang = 2 * np.pi * np.outer(n, cc) / 32.0
    m['dftC'] = np.cos(ang).astype(np.float16)
    m['dftS'] = (-np.sin(ang)).astype(np.float16)
    wc = np.where((cc == 0) | (cc == NCF - 1), 1.0, 2.0)
    angi = 2 * np.pi * np.outer(cc, n) / 32.0
    m['dftCi'] = (wc[:, None] * np.cos(angi) / 32.0).astype(np.float16)
    m['dftSi'] = (-wc[:, None] * np.sin(angi) / 32.0).astype(np.float16)

    m['wup'] = np.ascontiguousarray(np.broadcast_to(
        ((np.arange(UPF, dtype=np.float32) + 0.5) / UPF)[None, :], (128, UPF)))
    m['ones16'] = np.ones((128, 1), np.float16)
    stops = np.geomspace(20.0, SR / 2.0, num=NOSC)
    diffs = np.diff(np.concatenate([[0.0], stops]))
    m['kst'] = np.ascontiguousarray((stops - diffs).astype(np.float32).reshape(128, 1))
    m['kdf'] = np.ascontiguousarray(diffs.astype(np.float32).reshape(128, 1))
    return m


def _prep_core(x2, noise2):
    m = {}
    xp = np.zeros((128, TW), np.float32)
    xp[:, 1:65] = x2[0]
    xp[:, 67:131] = x2[1]
    hi, lo = _split16(xp)
    m['xh'] = hi
    m['xl'] = lo
    padded = np.pad(noise2.astype(np.float32), ((0, 0), (0, 16)))
    fr = np.zeros((32, 2 * NFRM), np.float32)
    idx = np.arange(NFRM)[None, :] * 16 + np.arange(32)[:, None]   # [32, 1024]
    for b in range(BL):
        fr[:, b * NFRM:(b + 1) * NFRM] = padded[b][idx]
    m['fr'] = fr.astype(np.float16)
    return m


def kernel(**inputs):
    nc = _CACHE.get('nc')
    if nc is None:
        nc = _build()
        _CACHE['nc'] = nc
    shared = _prep_shared(inputs)
    x = np.asarray(inputs['x'], np.float32)
    noise = np.asarray(inputs['noise'], np.float32)
    in_maps = []
    for c in range(N_CORES):
        m = dict(shared)
        m.update(_prep_core(x[2 * c:2 * c + 2], noise[2 * c:2 * c + 2]))
        in_maps.append(m)
    r = run_bass_kernel_spmd(nc, in_maps, core_ids=list(range(N_CORES)))
    outs = [r.results[c]['out'] for c in range(N_CORES)]
    return np.concatenate(outs, axis=0).reshape(16, 1, OUT_T).astype(np.float32)


if __name__ == '__main__':
    _build()
    print("built ok")
